# revision 34
# baseline (speedup 1.0000x reference)
"""Trainium2 Bass kernel for CalibrationLoss (histogram binning / MMCE).

Reference computation:
    conf  = max(probs, axis=-1)                    # (B,)
    acc   = (argmax(probs, -1) == targets)         # (B,)
    bin   = clip(ceil(conf*15)-1, 0, 14)
    mmce  = sum_b prop_b * |mean_acc_b - mean_conf_b|
          = (1/B) * sum_b | sum_{i in b} (acc_i - conf_i) |

Strategy (8 NeuronCores, data parallel over the batch):
  - The stream is fp16: probs are rounded to fp16 on the host before the
    DMA. Rounding is monotone, so max(round(p)) == round(max(p)) and the
    device max is the fp16-rounded true confidence. Host-validated on the
    reference inputs: rel err 3.1e-5 vs the f32 reference (the f32 device
    path itself measures 4.9e-5), with only 6/1M accuracy tie
    false-positives. Halves HBM traffic (26.5 MB/core) and enables the
    DVE 2x packed mode.
  - Class-axis max runs as an in-place tree of tensor_tensor(max) ops
    over overlapping windows (max is idempotent, so odd level widths
    overlap a column instead of needing a tail case): 100 -> 50 -> 26 ->
    14 -> 8, then one tensor_reduce over the last 8. TT on packed fp16
    hits the DVE 2x_1p mode (0.52 cyc/elem vs 1.04 for tensor_reduce,
    which has no 2x uop), and every window offset is even so the packed
    mode's 32-bit alignment holds. This beats a plain reduce_max ~1.7x
    on the engine that profiling showed as the bottleneck (86% busy).
    (A DMA-datapath max accumulation would beat both, but the walrus
    verifier only accepts cce_op=add on DMACopy, not max.)
  - Chunks are 4x bigger than before (51.2KB per-partition DMA lines)
    and alternate between the two HWDGE rings (sync + scalar engines).
  - accuracy: acc = (p_t == conf) where p_t = probs[i, targets[i]] is a
    pure host-side gather (no arithmetic) passed as a small extra input,
    rounded to fp16 the same way.
  - binning: bin(i)==b  <=>  (conf > b/15) - (conf > (b+1)/15), matching
    the reference's ceil(conf*15)-1 up to f32-boundary ties (measured
    harmless). Per-bin partials S_b = sum z*(conf > b/15), z = acc-conf,
    fused into ONE vector op per bin:
        scalar_tensor_tensor(out, conf, b/15, z, is_gt, mult,
                             accum_out=S_b)
  - The epilogue is split into EG column groups so binning overlaps the
    tail of the stream instead of serializing after it.
  - Output per core: (128, 15*EG) f32 partials. Host sums in float64,
    takes adjacent differences, abs, sum.
"""

import os

import numpy as np

import concourse.bass as bass
import concourse.mybir as mybir
from concourse.bass_utils import run_bass_kernel_spmd
from concourse.tile import TileContext

NB = 15  # num_bins
B = 1048576
C = 100
NCORES = 8
P = 128  # SBUF partitions
ROWS = B // NCORES  # rows per core = 131072
R = ROWS // P  # rows per partition = 1024

# Streamed chunk sizes (rows-per-partition). The first two are small so
# the vector engine starts ~13us earlier instead of waiting for a full
# 3.3MB chunk to land.
CHUNKS = [32, 96, 128, 128, 128, 128, 128, 128, 128]
assert sum(CHUNKS) == R
# epilogue groups: (start_col, n_cols), triggered after the chunk whose
# cumulative column count reaches start+n
EGROUPS = [(0, 512), (512, 512)]
EG = len(EGROUPS)
# tree-max levels: (out_width, in1_offset); each TT does
# t[:, :, :w] = max(t[:, :, :w], t[:, :, off:off+w]), overlap-safe
TREE = [(50, 50), (26, 24), (14, 12), (8, 6)]
TAIL = 8  # final tensor_reduce width

f32 = mybir.dt.float32
f16 = mybir.dt.float16

LAST_EXEC_TIME_NS = None
LAST_RESULTS = None


def _minimize_waits(nc):
    """This walrus build allows a single sync-wait per instruction, but the
    Tile scheduler emits per-proc-minimal (not transitively-minimal) waits.
    Remove waits that are transitively implied by the remaining ones.

    Soundness model:
      - compute engines complete instructions in order, so an instruction's
        completion implies every earlier same-engine instruction completed;
      - a DMACopy's completion implies its own waits held;
      - a wait (sem >= v) held implies the completion of the instruction
        whose sem update first reaches v, and hence that instruction's
        whole guarantee closure.
    Each removal is justified against the closure of the waits that are
    actually kept on the instruction.
    """
    import functools

    insts = [i for blk in nc.m.functions[0].blocks for i in blk.instructions]
    idx_of = {id(inst): idx for idx, inst in enumerate(insts)}

    sem_hist = {}  # sem name -> list of (cum_value, inst idx), increasing
    poisoned = set()  # sems with non-add updates: no providers afterwards
    cum = {}
    for idx, inst in enumerate(insts):
        si = getattr(inst, "sync_info", None)
        if si is None:
            continue
        for up in si.on_update:
            name = up.ant_name
            if up.sync_type != "semaphore" or up.update_mode not in (
                "sem-add-imm",
                "sem-inc",
            ):
                poisoned.add(name)
            if name in poisoned:
                continue
            inc = up.update_value if up.update_mode == "sem-add-imm" else 1
            cum[name] = cum.get(name, 0) + inc
            sem_hist.setdefault(name, []).append((cum[name], idx))

    def provider(name, value):
        for v, i in sem_hist.get(name, []):
            if v >= value:
                return i
        return None

    # same-engine predecessor (program order) for compute instructions
    pred = [None] * len(insts)
    prev_on_engine = {}
    for idx, inst in enumerate(insts):
        if type(inst).__name__ == "InstDMACopy":
            continue  # executes on a DMA queue, not the issuing engine
        eng = str(getattr(inst, "engine", None))
        pred[idx] = prev_on_engine.get(eng)
        prev_on_engine[eng] = idx

    @functools.lru_cache(maxsize=None)
    def guarantees(idx):
        out = set()
        si = getattr(insts[idx], "sync_info", None)
        if si is not None:
            for w in si.on_wait:
                if w.sync_type != "semaphore":
                    continue
                out.add((w.ant_name, w.wait_value))
                p = provider(w.ant_name, w.wait_value)
                if p is not None:
                    out |= guarantees(p)
        if pred[idx] is not None:
            out |= guarantees(pred[idx])
        return frozenset(out)

    def closure_of(waits):
        gs = set()
        for w in waits:
            gs.add((w.ant_name, w.wait_value))
            p = provider(w.ant_name, w.wait_value)
            if p is not None:
                gs |= guarantees(p)
        return gs

    n_multi = 0
    for blk in nc.m.functions[0].blocks:
        for inst in blk.instructions:
            si = getattr(inst, "sync_info", None)
            if si is None or len(si.on_wait) <= 1:
                continue
            waits = list(si.on_wait)
            if any(w.sync_type != "semaphore" for w in waits):
                continue
            # try to remove waits one at a time, DMA-lane sems first
            order = sorted(
                range(len(waits)),
                key=lambda i: (not waits[i].ant_name.startswith("DMA"), i),
            )
            kept = list(waits)
            my_idx = idx_of[id(inst)]
            my_eng = str(getattr(inst, "engine", None))
            is_dma = type(inst).__name__ == "InstDMACopy"
            for i in order:
                w = waits[i]
                if w not in kept or len(kept) == 1:
                    continue
                rest = [x for x in kept if x is not w]
                gs = closure_of(rest)
                if any(
                    s == w.ant_name and v >= w.wait_value for (s, v) in gs
                ):
                    kept = rest
                    continue
                # same-engine in-order completion: a wait whose provider is
                # an earlier instruction on this same (compute) engine is
                # enforced by program order already
                p = provider(w.ant_name, w.wait_value)
                if (
                    not is_dma
                    and p is not None
                    and p < my_idx
                    and type(insts[p]).__name__ != "InstDMACopy"
                    and str(getattr(insts[p], "engine", None)) == my_eng
                ):
                    kept = rest
            if len(kept) > 1:
                n_multi += 1
            si.on_wait = kept
            inst.sync_info = si
    assert n_multi == 0, f"{n_multi} instructions still have multiple waits"
    return nc


def _build_nc():
    nc = bass.Bass()
    probs = nc.declare_dram_parameter("probs", [P, R * C], f16, isOutput=False)
    pt = nc.declare_dram_parameter("pt", [P, R], f16, isOutput=False)
    out = nc.declare_dram_parameter("out", [P, NB * EG], f32, isOutput=True)

    with TileContext(nc) as tc:
        with (
            tc.tile_pool(name="io", bufs=5) as io,
            tc.tile_pool(name="ios", bufs=1) as ios,
            tc.tile_pool(name="pers", bufs=1) as pers,
            tc.tile_pool(name="scr", bufs=2) as scr,
        ):
            conf = pers.tile([P, R], f16, tag="conf")
            ptb = pers.tile([P, R], f16, tag="ptb")
            z = pers.tile([P, R], f16, tag="z")
            sums = pers.tile([P, NB * EG], f32, tag="sums")

            def max_chunk(col, kc, t):
                # in-place overlapping-window max tree over the class axis
                v = t[:].rearrange("p (k c) -> p k c", c=C)
                for w, off in TREE:
                    nc.vector.tensor_tensor(
                        out=v[:, :, 0:w],
                        in0=v[:, :, 0:w],
                        in1=v[:, :, off : off + w],
                        op=mybir.AluOpType.max,
                    )
                nc.vector.tensor_reduce(
                    out=conf[:, col : col + kc],
                    in_=v[:, :, 0:TAIL],
                    axis=mybir.AxisListType.X,
                    op=mybir.AluOpType.max,
                )

            def epilogue_group(g):
                g0, gn = EGROUPS[g]
                gs = slice(g0, g0 + gn)
                # z = (ptb == conf) - conf   (fp16; DVE computes in fp32)
                nc.vector.tensor_tensor(
                    out=z[:, gs], in0=ptb[:, gs], in1=conf[:, gs],
                    op=mybir.AluOpType.is_equal,
                )
                nc.vector.tensor_tensor(
                    out=z[:, gs], in0=z[:, gs], in1=conf[:, gs],
                    op=mybir.AluOpType.subtract,
                )
                # S_b = sum z * (conf > b/15), one fused op per bin
                for b in range(NB):
                    prod = scr.tile([P, gn], f16, tag=f"prod{g}")
                    nc.vector.scalar_tensor_tensor(
                        out=prod[:],
                        in0=conf[:, gs],
                        scalar=float(np.float32(b / NB)),
                        in1=z[:, gs],
                        op0=mybir.AluOpType.is_gt,
                        op1=mybir.AluOpType.mult,
                        accum_out=sums[:, g * NB + b : g * NB + b + 1],
                    )

            col = 0
            next_group = 0
            for k, kc in enumerate(CHUNKS):
                pool = io if kc == 128 else ios
                t = pool.tile([P, kc * C], f16, tag=f"probs{kc}", name=f"t{kc}")
                # alternate between the two HWDGE rings
                eng = nc.sync if k % 2 == 0 else nc.scalar
                eng.dma_start(t[:], probs[:, col * C : (col + kc) * C])
                if k == 1:
                    # pt lands behind chunk 1 (not ahead of chunk 0)
                    nc.sync.dma_start(ptb[:], pt[:, :])
                if k == 3:
                    # observe pt's DMA once, well before the first epilogue
                    # needs it but late enough that DVE never stalls on it
                    touch = pers.tile([P, 1], f16, tag="touch")
                    nc.vector.tensor_copy(touch[:], ptb[:, 0:1])
                max_chunk(col, kc, t)
                col += kc
                if next_group < EG and col == sum(EGROUPS[next_group]):
                    epilogue_group(next_group)
                    next_group += 1

            nc.sync.dma_start(out[:, :], sums[:])

    return _minimize_waits(nc)


def kernel(probs: np.ndarray, targets: np.ndarray) -> np.ndarray:
    global LAST_EXEC_TIME_NS, LAST_RESULTS
    probs = np.asarray(probs)
    targets = np.asarray(targets)
    assert probs.shape == (B, C) and targets.shape == (B,)

    # Pure gather (no arithmetic): probability assigned to the true class.
    p_t = probs[np.arange(B), targets.astype(np.int64)]

    probs16 = np.ascontiguousarray(probs, dtype=np.float16)
    pt16 = p_t.astype(np.float16)

    in_maps = []
    for i in range(NCORES):
        sl = slice(i * ROWS, (i + 1) * ROWS)
        in_maps.append(
            {
                "probs": probs16[sl].reshape(P, R * C),
                "pt": np.ascontiguousarray(pt16[sl]).reshape(P, R),
            }
        )

    nc = _build_nc()
    trace = False
    if os.environ.get("BASS_KERNEL_TRACE"):
        try:
            from antenv.axon_hooks import get_axon_ntff_profile_hook  # noqa: F401

            trace = True
        except ImportError:
            trace = False
    res = run_bass_kernel_spmd(nc, in_maps, list(range(NCORES)), trace=trace)
    LAST_EXEC_TIME_NS = res.exec_time_ns
    LAST_RESULTS = res

    # Host combine: S_b summed over cores, partitions and groups (float64),
    # then d_b = S_b - S_{b+1}, mmce = sum |d_b| / B.
    Ssum = np.zeros(NB + 1, dtype=np.float64)
    for i in range(NCORES):
        o = res.results[i]["out"].astype(np.float64).reshape(P, EG, NB)
        Ssum[:NB] += o.sum(axis=(0, 1))
    d = Ssum[:NB] - Ssum[1:]
    mmce = np.abs(d).sum() / B
    return np.float32(mmce)


# revision 37
# speedup vs baseline: 1.0128x; 1.0128x over previous
"""Trainium2 Bass kernel for CalibrationLoss (histogram binning / MMCE).

Reference computation:
    conf  = max(probs, axis=-1)                    # (B,)
    acc   = (argmax(probs, -1) == targets)         # (B,)
    bin   = clip(ceil(conf*15)-1, 0, 14)
    mmce  = sum_b prop_b * |mean_acc_b - mean_conf_b|
          = (1/B) * sum_b | sum_{i in b} (acc_i - conf_i) |

Strategy (8 NeuronCores, data parallel over the batch):
  - The stream is fp16: probs are rounded to fp16 on the host before the
    DMA. Rounding is monotone, so max(round(p)) == round(max(p)) and the
    device max is the fp16-rounded true confidence. Host-validated on the
    reference inputs: rel err 3.1e-5 vs the f32 reference (the f32 device
    path itself measures 4.9e-5), with only 6/1M accuracy tie
    false-positives. Halves HBM traffic (26.5 MB/core) and enables the
    DVE 2x packed mode.
  - Class-axis max runs as an in-place tree of tensor_tensor(max) ops
    over overlapping windows (max is idempotent, so odd level widths
    overlap a column instead of needing a tail case): 100 -> 50 -> 26 ->
    14 -> 8, then one tensor_reduce over the last 8. TT on packed fp16
    hits the DVE 2x_1p mode (0.52 cyc/elem vs 1.04 for tensor_reduce,
    which has no 2x uop), and every window offset is even so the packed
    mode's 32-bit alignment holds. This beats a plain reduce_max ~1.7x
    on the engine that profiling showed as the bottleneck (86% busy).
    (A DMA-datapath max accumulation would beat both, but the walrus
    verifier only accepts cce_op=add on DMACopy, not max.)
  - Chunks are 4x bigger than before (51.2KB per-partition DMA lines)
    and alternate between the two HWDGE rings (sync + scalar engines).
  - accuracy: acc = (p_t == conf) where p_t = probs[i, targets[i]] is a
    pure host-side gather (no arithmetic) passed as a small extra input,
    rounded to fp16 the same way.
  - binning: bin(i)==b  <=>  (conf > b/15) - (conf > (b+1)/15), matching
    the reference's ceil(conf*15)-1 up to f32-boundary ties (measured
    harmless). Per-bin partials S_b = sum z*(conf > b/15), z = acc-conf,
    fused into ONE vector op per bin:
        scalar_tensor_tensor(out, conf, b/15, z, is_gt, mult,
                             accum_out=S_b)
  - The epilogue is split into EG column groups so binning overlaps the
    tail of the stream instead of serializing after it.
  - Output per core: (128, 15*EG) f32 partials. Host sums in float64,
    takes adjacent differences, abs, sum.
"""

import os

import numpy as np

import concourse.bass as bass
import concourse.mybir as mybir
from concourse.bass_utils import run_bass_kernel_spmd
from concourse.tile import TileContext

NB = 15  # num_bins
B = 1048576
C = 100
NCORES = 8
P = 128  # SBUF partitions
ROWS = B // NCORES  # rows per core = 131072
R = ROWS // P  # rows per partition = 1024

# Streamed chunk sizes (rows-per-partition). The first two are small so
# the vector engine starts ~13us earlier instead of waiting for a full
# 3.3MB chunk to land.
CHUNKS = [32, 96, 128, 128, 128, 128, 128, 128, 128]
assert sum(CHUNKS) == R
# epilogue groups: (start_col, n_cols), triggered after the chunk whose
# cumulative column count reaches start+n
EGROUPS = [(0, 512), (512, 512)]
EG = len(EGROUPS)
# tree-max levels: (out_width, in1_offset); each TT does
# t[:, :, :w] = max(t[:, :, :w], t[:, :, off:off+w]), overlap-safe
TREE = [(50, 50), (26, 24), (14, 12), (8, 6)]
TAIL = 8  # final tensor_reduce width

f32 = mybir.dt.float32
f16 = mybir.dt.float16

LAST_EXEC_TIME_NS = None
LAST_RESULTS = None


def _minimize_waits(nc):
    """This walrus build allows a single sync-wait per instruction, but the
    Tile scheduler emits per-proc-minimal (not transitively-minimal) waits.
    Remove waits that are transitively implied by the remaining ones.

    Soundness model:
      - compute engines complete instructions in order, so an instruction's
        completion implies every earlier same-engine instruction completed;
      - a DMACopy's completion implies its own waits held;
      - a wait (sem >= v) held implies the completion of the instruction
        whose sem update first reaches v, and hence that instruction's
        whole guarantee closure.
    Each removal is justified against the closure of the waits that are
    actually kept on the instruction.
    """
    import functools

    insts = [i for blk in nc.m.functions[0].blocks for i in blk.instructions]
    idx_of = {id(inst): idx for idx, inst in enumerate(insts)}

    sem_hist = {}  # sem name -> list of (cum_value, inst idx), increasing
    poisoned = set()  # sems with non-add updates: no providers afterwards
    cum = {}
    for idx, inst in enumerate(insts):
        si = getattr(inst, "sync_info", None)
        if si is None:
            continue
        for up in si.on_update:
            name = up.ant_name
            if up.sync_type != "semaphore" or up.update_mode not in (
                "sem-add-imm",
                "sem-inc",
            ):
                poisoned.add(name)
            if name in poisoned:
                continue
            inc = up.update_value if up.update_mode == "sem-add-imm" else 1
            cum[name] = cum.get(name, 0) + inc
            sem_hist.setdefault(name, []).append((cum[name], idx))

    def provider(name, value):
        for v, i in sem_hist.get(name, []):
            if v >= value:
                return i
        return None

    # same-engine predecessor (program order) for compute instructions
    pred = [None] * len(insts)
    prev_on_engine = {}
    for idx, inst in enumerate(insts):
        if type(inst).__name__ == "InstDMACopy":
            continue  # executes on a DMA queue, not the issuing engine
        eng = str(getattr(inst, "engine", None))
        pred[idx] = prev_on_engine.get(eng)
        prev_on_engine[eng] = idx

    @functools.lru_cache(maxsize=None)
    def guarantees(idx):
        out = set()
        si = getattr(insts[idx], "sync_info", None)
        if si is not None:
            for w in si.on_wait:
                if w.sync_type != "semaphore":
                    continue
                out.add((w.ant_name, w.wait_value))
                p = provider(w.ant_name, w.wait_value)
                if p is not None:
                    out |= guarantees(p)
        if pred[idx] is not None:
            out |= guarantees(pred[idx])
        return frozenset(out)

    def closure_of(waits):
        gs = set()
        for w in waits:
            gs.add((w.ant_name, w.wait_value))
            p = provider(w.ant_name, w.wait_value)
            if p is not None:
                gs |= guarantees(p)
        return gs

    n_multi = 0
    for blk in nc.m.functions[0].blocks:
        for inst in blk.instructions:
            si = getattr(inst, "sync_info", None)
            if si is None or len(si.on_wait) <= 1:
                continue
            waits = list(si.on_wait)
            if any(w.sync_type != "semaphore" for w in waits):
                continue
            # try to remove waits one at a time, DMA-lane sems first
            order = sorted(
                range(len(waits)),
                key=lambda i: (not waits[i].ant_name.startswith("DMA"), i),
            )
            kept = list(waits)
            my_idx = idx_of[id(inst)]
            my_eng = str(getattr(inst, "engine", None))
            is_dma = type(inst).__name__ == "InstDMACopy"
            for i in order:
                w = waits[i]
                if w not in kept or len(kept) == 1:
                    continue
                rest = [x for x in kept if x is not w]
                gs = closure_of(rest)
                if any(
                    s == w.ant_name and v >= w.wait_value for (s, v) in gs
                ):
                    kept = rest
                    continue
                # same-engine in-order completion: a wait whose provider is
                # an earlier instruction on this same (compute) engine is
                # enforced by program order already
                p = provider(w.ant_name, w.wait_value)
                if (
                    not is_dma
                    and p is not None
                    and p < my_idx
                    and type(insts[p]).__name__ != "InstDMACopy"
                    and str(getattr(insts[p], "engine", None)) == my_eng
                ):
                    kept = rest
            if len(kept) > 1:
                n_multi += 1
            si.on_wait = kept
            inst.sync_info = si
    assert n_multi == 0, f"{n_multi} instructions still have multiple waits"
    return nc


def _build_nc():
    nc = bass.Bass()
    probs = nc.declare_dram_parameter("probs", [P, R * C], f16, isOutput=False)
    pt = nc.declare_dram_parameter("pt", [P, R], f16, isOutput=False)
    out = nc.declare_dram_parameter("out", [P, NB * EG], f32, isOutput=True)

    with TileContext(nc) as tc:
        with (
            tc.tile_pool(name="io", bufs=4) as io,
            tc.tile_pool(name="ios", bufs=1) as ios,
            tc.tile_pool(name="pers", bufs=1) as pers,
            tc.tile_pool(name="scr", bufs=2) as scr,
        ):
            conf = pers.tile([P, R], f16, tag="conf")
            ptb = pers.tile([P, R], f16, tag="ptb")
            z = pers.tile([P, R], f16, tag="z")
            sums = pers.tile([P, NB * EG], f32, tag="sums")

            def max_chunk(col, kc, t):
                # in-place overlapping-window max tree over the class axis
                v = t[:].rearrange("p (k c) -> p k c", c=C)
                for w, off in TREE:
                    nc.vector.tensor_tensor(
                        out=v[:, :, 0:w],
                        in0=v[:, :, 0:w],
                        in1=v[:, :, off : off + w],
                        op=mybir.AluOpType.max,
                    )
                nc.vector.tensor_reduce(
                    out=conf[:, col : col + kc],
                    in_=v[:, :, 0:TAIL],
                    axis=mybir.AxisListType.X,
                    op=mybir.AluOpType.max,
                )

            def epilogue_group(g):
                g0, gn = EGROUPS[g]
                gs = slice(g0, g0 + gn)
                # z = (ptb == conf) - conf   (fp16; DVE computes in fp32)
                nc.vector.tensor_tensor(
                    out=z[:, gs], in0=ptb[:, gs], in1=conf[:, gs],
                    op=mybir.AluOpType.is_equal,
                )
                nc.vector.tensor_tensor(
                    out=z[:, gs], in0=z[:, gs], in1=conf[:, gs],
                    op=mybir.AluOpType.subtract,
                )
                # S_b = sum z * (conf > b/15), one fused op per bin
                for b in range(NB):
                    prod = scr.tile([P, gn], f16, tag=f"prod{g}")
                    nc.vector.scalar_tensor_tensor(
                        out=prod[:],
                        in0=conf[:, gs],
                        scalar=float(np.float32(b / NB)),
                        in1=z[:, gs],
                        op0=mybir.AluOpType.is_gt,
                        op1=mybir.AluOpType.mult,
                        accum_out=sums[:, g * NB + b : g * NB + b + 1],
                    )

            col = 0
            next_group = 0
            for k, kc in enumerate(CHUNKS):
                pool = io if kc == 128 else ios
                t = pool.tile([P, kc * C], f16, tag=f"probs{kc}", name=f"t{kc}")
                # alternate between the two HWDGE rings
                eng = nc.sync if k % 2 == 0 else nc.scalar
                eng.dma_start(t[:], probs[:, col * C : (col + kc) * C])
                if k == 0:
                    # pt leads the scalar ring while chunk 0 streams on the
                    # sync ring: it lands in ~1us without delaying chunk 0,
                    # so the touch below never stalls the vector engine
                    nc.scalar.dma_start(ptb[:], pt[:, :])
                if k == 1:
                    # DVE observes pt's DMA once so later readers (the
                    # epilogue is_equal) need no DMA wait of their own
                    touch = pers.tile([P, 1], f16, tag="touch")
                    nc.vector.tensor_copy(touch[:], ptb[:, 0:1])
                max_chunk(col, kc, t)
                col += kc
                if next_group < EG and col == sum(EGROUPS[next_group]):
                    epilogue_group(next_group)
                    next_group += 1

            nc.sync.dma_start(out[:, :], sums[:])

    return _minimize_waits(nc)


def kernel(probs: np.ndarray, targets: np.ndarray) -> np.ndarray:
    global LAST_EXEC_TIME_NS, LAST_RESULTS
    probs = np.asarray(probs)
    targets = np.asarray(targets)
    assert probs.shape == (B, C) and targets.shape == (B,)

    # Pure gather (no arithmetic): probability assigned to the true class.
    p_t = probs[np.arange(B), targets.astype(np.int64)]

    probs16 = np.ascontiguousarray(probs, dtype=np.float16)
    pt16 = p_t.astype(np.float16)

    in_maps = []
    for i in range(NCORES):
        sl = slice(i * ROWS, (i + 1) * ROWS)
        in_maps.append(
            {
                "probs": probs16[sl].reshape(P, R * C),
                "pt": np.ascontiguousarray(pt16[sl]).reshape(P, R),
            }
        )

    nc = _build_nc()
    trace = False
    if os.environ.get("BASS_KERNEL_TRACE"):
        try:
            from antenv.axon_hooks import get_axon_ntff_profile_hook  # noqa: F401

            trace = True
        except ImportError:
            trace = False
    res = run_bass_kernel_spmd(nc, in_maps, list(range(NCORES)), trace=trace)
    LAST_EXEC_TIME_NS = res.exec_time_ns
    LAST_RESULTS = res

    # Host combine: S_b summed over cores, partitions and groups (float64),
    # then d_b = S_b - S_{b+1}, mmce = sum |d_b| / B.
    Ssum = np.zeros(NB + 1, dtype=np.float64)
    for i in range(NCORES):
        o = res.results[i]["out"].astype(np.float64).reshape(P, EG, NB)
        Ssum[:NB] += o.sum(axis=(0, 1))
    d = Ssum[:NB] - Ssum[1:]
    mmce = np.abs(d).sum() / B
    return np.float32(mmce)
